# revision 2
# baseline (speedup 1.0000x reference)
"""Trainium2 Bass kernel for nn_CrossTransFormer_86526411145604.

Computation (b=4, C=1024, H=8 heads, dh=128, p=2048):
  Q = LeakyReLU(BN1(Wq @ Xq)), K = LeakyReLU(BN2(Wk @ Xk)), V = LeakyReLU(BN3(Wv @ Xq))
  per (b,h): S = Kh^T Vh / sqrt(dh); A = softmax_j(S); out[c,i] = sum_j A[i,j] Qh[c,j]

Sharding: 8 cores = (4 batches) x (2 head-groups of 4 heads). Each core gets
Xq[b], Xk[b] and the 512-channel slice of the (BN-scale-folded, transposed)
weights for its head group.

Structure (v2): head-interleaved schedule so the scalar engine's exp work
overlaps the PE's branch matmuls. The second attention matmul runs in
"form 1": out^T[i,c] = sum_j P^T[j,i] Qt[j,c] with the P^T chunk stationary
and a moving operand [Qt | 1] that carries a ones column, so the softmax
denominator accumulates as a free 129th output column (no row-sum matmuls,
no cross-partition broadcast). The per-head output is produced transposed
([i, c]); the host transposes it back when gathering.
"""

import math
import os

import numpy as np

C = 1024
H = 8
EPS = 1e-5
SLOPE = 0.1
B = 4
P = 2048
HG = 4            # heads per core
NKC = C // 128    # contraction chunks for the branch matmuls
NJC = P // 128    # j chunks for the attention contraction
NIC = 8           # i chunks (of 128) per 1024-wide half

_PROGRAM = None
LAST_RESULTS = None


def _build_program():
    import concourse.mybir as mybir
    import concourse.tile as tile
    from concourse import bacc

    f32 = mybir.dt.float32
    bf16 = mybir.dt.bfloat16
    LRELU = mybir.ActivationFunctionType.Prelu
    EXP = mybir.ActivationFunctionType.Exp

    nc = bacc.Bacc("TRN2", target_bir_lowering=False, debug=False)

    xq = nc.dram_tensor("xq", [C, P], bf16, kind="ExternalInput")
    xk = nc.dram_tensor("xk", [C, P], bf16, kind="ExternalInput")
    wq = nc.dram_tensor("wq", [C, 512], bf16, kind="ExternalInput")
    wk = nc.dram_tensor("wk", [C, 512], bf16, kind="ExternalInput")
    wv = nc.dram_tensor("wv", [C, 512], bf16, kind="ExternalInput")
    bq = nc.dram_tensor("bq", [1, 512], bf16, kind="ExternalInput")
    bk = nc.dram_tensor("bk", [128, HG], f32, kind="ExternalInput")
    bv = nc.dram_tensor("bv", [128, HG], f32, kind="ExternalInput")
    out_t = nc.dram_tensor("out_t", [P, 512], f32, kind="ExternalOutput")

    sc = 1.0 / math.sqrt(C / H)

    with tile.TileContext(nc) as tc:
        with tc.tile_pool(name="wpool", bufs=1) as wpool, \
             tc.tile_pool(name="cpool", bufs=1) as cpool, \
             tc.tile_pool(name="kvpool", bufs=2) as kvpool, \
             tc.tile_pool(name="ptpool", bufs=18) as ptpool, \
             tc.tile_pool(name="opool", bufs=4) as opool, \
             tc.tile_pool(name="pq", bufs=2, space="PSUM") as pq, \
             tc.tile_pool(name="ps", bufs=2, space="PSUM") as ps, \
             tc.tile_pool(name="po", bufs=2, space="PSUM") as po:

            wq_sb = wpool.tile([128, NKC, 512], bf16)
            wk_sb = wpool.tile([128, NKC, 512], bf16)
            wv_sb = wpool.tile([128, NKC, 512], bf16)
            xq_sb = wpool.tile([128, NKC, P], bf16)
            xk_sb = wpool.tile([128, NKC, P], bf16)
            bq_sb = cpool.tile([1, 512], bf16)
            bk_sb = cpool.tile([128, HG], f32)
            bv_sb = cpool.tile([128, HG], f32)
            ones_row = cpool.tile([1, 128], bf16)
            nc.vector.memset(ones_row[:], 1.0)

            # qt holds [Qt | 1] per (j-chunk, head): 129 columns each.
            qt_sb = wpool.tile([128, NJC, HG, 129], bf16)
            for hl in range(HG):
                nc.vector.memset(qt_sb[:, :, hl, 128:129], 1.0)

            xqv = xq.ap().rearrange("(kc p) i -> p kc i", p=128)
            xkv = xk.ap().rearrange("(kc p) i -> p kc i", p=128)

            def _load_w(wsb, wdr):
                wview = wdr.ap().rearrange("(kc p) n -> p kc n", p=128)
                for half in range(2):
                    hs = slice(half * NKC // 2, (half + 1) * NKC // 2)
                    nc.sync.dma_start(wsb[:, hs, :], wview[:, hs, :])

            # Load order = first-use order: Q phase (wq, xq), V branch (wv),
            # K branch (wk, xk).
            _load_w(wq_sb, wq)
            nc.sync.dma_start(bq_sb[:], bq.ap())
            for t in range(4):
                cs = slice(t * 512, (t + 1) * 512)
                nc.sync.dma_start(xq_sb[:, :, cs], xqv[:, :, cs])
            _load_w(wv_sb, wv)
            nc.sync.dma_start(bv_sb[:], bv.ap())
            _load_w(wk_sb, wk)
            nc.sync.dma_start(bk_sb[:], bk.ap())
            for t in range(4):
                cs = slice(t * 512, (t + 1) * 512)
                nc.sync.dma_start(xk_sb[:, :, cs], xkv[:, :, cs])

            # ---- Q branch (all heads): qt[j, c] with trailing ones cols ----
            for js in range(NJC):
                ps_q = pq.tile([128, 512], f32, tag="mm")
                for kc in range(NKC):
                    nc.tensor.matmul(ps_q[:], xq_sb[:, kc, js * 128:(js + 1) * 128],
                                     wq_sb[:, kc, :],
                                     start=(kc == 0), stop=False)
                nc.tensor.matmul(ps_q[:], ones_row[:, 0:128], bq_sb[:],
                                 start=False, stop=True)
                nc.scalar.activation(qt_sb[:, js, :, 0:128], ps_q[:], LRELU,
                                     alpha=SLOPE)

            # ---- per head: V/K branch, S^T + exp, out^T + normalize ----
            for hl in range(HG):
                hs = slice(hl * 128, (hl + 1) * 128)
                vh_sb = kvpool.tile([128, P], bf16, tag="vh")
                kh_sb = kvpool.tile([128, P], bf16, tag="kh")
                for t in range(4):
                    cs = slice(t * 512, (t + 1) * 512)
                    ps_v = pq.tile([128, 512], f32, tag="mm")
                    for kc in range(NKC):
                        nc.tensor.matmul(ps_v[:], wv_sb[:, kc, hs],
                                         xq_sb[:, kc, cs],
                                         start=(kc == 0), stop=(kc == NKC - 1))
                    nc.scalar.activation(vh_sb[:, cs], ps_v[:], LRELU,
                                         bias=bv_sb[:, hl:hl + 1], alpha=SLOPE)
                for t in range(4):
                    cs = slice(t * 512, (t + 1) * 512)
                    ps_k = pq.tile([128, 512], f32, tag="mm")
                    for kc in range(NKC):
                        nc.tensor.matmul(ps_k[:], wk_sb[:, kc, hs],
                                         xk_sb[:, kc, cs],
                                         start=(kc == 0), stop=(kc == NKC - 1))
                    nc.scalar.activation(kh_sb[:, cs], ps_k[:], LRELU,
                                         bias=bk_sb[:, hl:hl + 1], alpha=SLOPE)

                for ih in range(2):
                    pts = []
                    for jm in range(NJC):
                        ps_s = ps.tile([128, 1024], f32, tag="mm")
                        for sub in range(2):
                            nc.tensor.matmul(
                                ps_s[:, sub * 512:(sub + 1) * 512],
                                vh_sb[:, jm * 128:(jm + 1) * 128],
                                kh_sb[:, ih * 1024 + sub * 512:
                                      ih * 1024 + (sub + 1) * 512],
                                start=True, stop=True)
                        pt = ptpool.tile([128, 1024], bf16, tag="pt")
                        nc.scalar.activation(pt[:], ps_s[:], EXP, scale=sc)
                        pts.append(pt)
                    for ic in range(NIC):
                        po_t = po.tile([128, 132], f32, tag="po")
                        for jm in range(NJC):
                            nc.tensor.matmul(
                                po_t[:, 0:129],
                                pts[jm][:, ic * 128:(ic + 1) * 128],
                                qt_sb[:, jm, hl, :],
                                start=(jm == 0), stop=(jm == NJC - 1))
                        r = opool.tile([128, 1], f32, tag="r")
                        nc.vector.reciprocal_approx_fast(r[:], po_t[:, 128:129])
                        ot = opool.tile([128, 128], f32, tag="ot")
                        nc.vector.tensor_scalar_mul(ot[:], po_t[:, 0:128], r[:])
                        i0 = ih * 1024 + ic * 128
                        nc.sync.dma_start(out_t.ap()[i0:i0 + 128, hs], ot[:])

    nc.compile()
    return nc


def _get_program():
    global _PROGRAM
    if _PROGRAM is None:
        _PROGRAM = _build_program()
    return _PROGRAM


def kernel(Xq, Xk, Wq, Wk, Wv,
           gamma1, beta1, mean1, var1,
           gamma2, beta2, mean2, var2,
           gamma3, beta3, mean3, var3):
    global LAST_RESULTS
    from concourse.bass_utils import run_bass_kernel_spmd

    Xq = np.asarray(Xq, np.float32)
    Xk = np.asarray(Xk, np.float32)

    def fold(Wm, gamma, beta, mean, var):
        scale = np.asarray(gamma, np.float32) / np.sqrt(np.asarray(var, np.float32) + EPS)
        bias = np.asarray(beta, np.float32) - np.asarray(mean, np.float32) * scale
        Ws = np.asarray(Wm, np.float32) * scale[:, None]
        return Ws, bias

    Wq_s, b1 = fold(Wq, gamma1, beta1, mean1, var1)
    Wk_s, b2 = fold(Wk, gamma2, beta2, mean2, var2)
    Wv_s, b3 = fold(Wv, gamma3, beta3, mean3, var3)

    import ml_dtypes
    bf = ml_dtypes.bfloat16
    Xq_b = Xq.astype(bf)
    Xk_b = Xk.astype(bf)
    in_maps = []
    for core in range(8):
        b, hg = divmod(core, 2)
        sl = slice(hg * 512, (hg + 1) * 512)
        in_maps.append({
            "xq": np.ascontiguousarray(Xq_b[b]),
            "xk": np.ascontiguousarray(Xk_b[b]),
            "wq": np.ascontiguousarray(Wq_s[sl, :].T.astype(bf)),
            "wk": np.ascontiguousarray(Wk_s[sl, :].T.astype(bf)),
            "wv": np.ascontiguousarray(Wv_s[sl, :].T.astype(bf)),
            "bq": np.ascontiguousarray(b1[sl].reshape(1, 512).astype(bf)),
            "bk": np.ascontiguousarray(b2[sl].reshape(HG, 128).T),
            "bv": np.ascontiguousarray(b3[sl].reshape(HG, 128).T),
        })

    nc = _get_program()
    trace = os.environ.get("KERNEL_TRACE", "0") == "1"
    res = run_bass_kernel_spmd(nc, in_maps, core_ids=list(range(8)), trace=trace)
    LAST_RESULTS = res

    full = np.empty((B, C, P), np.float32)
    for core in range(8):
        b, hg = divmod(core, 2)
        full[b, hg * 512:(hg + 1) * 512, :] = res.results[core]["out_t"].T
    return full
